# revision 8
# baseline (speedup 1.0000x reference)
"""Trainium2 Bass kernel for nn_BandSplitDCTFilter.

Math: the reference's mirror-FFT DCT / band filter / inverse collapses to
    out_c = C1 (Z_c) C2^T - S1 (Z_c) S2^T,   Z_c = (A x_c A^T) .* W_eff_c
with A[k,j] = 2cos(pi k (2j+1)/128); C2/S2 carry the irfft half-spectrum
weights u_l and the 1/(4HW) scale; W_eff = pad(W_low)+pad(W_mid)+W_high
merges the three bands (they share the inverse basis under zero-padding).
Then y = x_out @ proj_w^T and LayerNorm.

Sharding: pure data-parallel, one sample per core (B=8 = 8 cores), small
weights replicated.

v11: the two DRAM layout pivots (k<->w after the row DCT, n<->k between
the two inverse transforms) dominate when runs are 256B (128ch x bf16):
the DMA cost model doubles sub-512B descriptors. Fix: split the FRONT
pipes by w-half instead of channel-half so all 256 channels stay
contiguous in the free dim -> every pivot descriptor is 512B and
descriptor count halves. From the w-DCT on, the stream is merged
(full 128-partition matmuls, c256 free). The tail (second pivot load /
inverse-k / proj / LN / store) streams per 16-column quarter so pivot
DMA overlaps projection. All data bf16 except PSUM and LN stats; the
host up-casts y.
"""

import os

os.environ.setdefault("JAX_PLATFORMS", "axon,cpu")

import numpy as np
import ml_dtypes

import bass_rust
import concourse.bass as bass
import concourse.mybir as mybir
from concourse.tile import TileContext, ScopedClock
from concourse.bass_utils import run_bass_kernel_spmd

# ---------------------------------------------------------------------------
# Workarounds: this container's walrus rejects >1 sync wait per instruction.
# ---------------------------------------------------------------------------

_wait_ctr = 0


def _split_multi_waits(nc, max_waits=1):
    global _wait_ctr
    for f in nc.m.functions:
        for bb in f.blocks:
            out = []
            dirty = False
            for ins in bb.instructions:
                si = ins.sync_info
                if si is not None and len(si.on_wait) > max_waits:
                    waits = list(si.on_wait)
                    for w in waits[:-max_waits]:
                        _wait_ctr += 1
                        nop = bass_rust.InstNoOp(name=f"I-waitsplit-{_wait_ctr}")
                        nop.engine = ins.engine
                        nop.sync_info = mybir.SyncInfo(on_wait=[w], on_update=[])
                        out.append(nop)
                    ins.sync_info = mybir.SyncInfo(
                        on_wait=waits[-max_waits:], on_update=list(si.on_update)
                    )
                    dirty = True
                out.append(ins)
            if dirty:
                bb.instructions = out


def _patched_drain_and_barrier(self, tick_clock, wait_clock):
    nc = self.nc
    probe = nc.sync.nop(nofuse=True)
    wait_clock.add_sem_waits(probe.ins, ScopedClock({None: tick_clock.global_clock}))
    si = probe.ins.sync_info
    waits = list(si.on_wait) if si is not None else []
    probe.ins.sync_info = mybir.SyncInfo(on_wait=waits[:1], on_update=[])
    name2sem = {s.name: s for s in self.sems.allocated().values()}
    for w in waits[1:]:
        nc.sync.nop(nofuse=True)._wait_ge(name2sem[w.ant_name], w.wait_value)
    nc.sync.drain()
    nc.all_engine_barrier()
    popped = nc._tile_sem_poison_stack.pop()
    assert popped is self._sem_poison
    nc.clear_and_free_semaphores(list(self.sems.allocated().values()))
    nc.all_engine_barrier()


TileContext._drain_and_barrier = _patched_drain_and_barrier

# ---------------------------------------------------------------------------

B, H, W, C = 8, 64, 64, 256
N = H * W
F32 = mybir.dt.float32
BF16 = mybir.dt.bfloat16
ALU = mybir.AluOpType
ACTF = mybir.ActivationFunctionType


def _host_matrices():
    k = np.arange(64)
    j = np.arange(64)
    ang = np.pi * k[:, None] * (2 * j[None, :] + 1) / 128.0
    A = 2.0 * np.cos(ang)
    u = np.where(k == 0, 1.0, 2.0)
    C1T = np.cos(ang)
    S1T = np.sin(ang)
    C2T = u[:, None] * np.cos(ang) / 16384.0
    S2T = u[:, None] * np.sin(ang) / 16384.0

    AT = A.T.astype(np.float32)                                   # [h, k]
    khbd = np.zeros((128, 128), np.float32)
    khbd[0:64, 0:64] = AT
    khbd[64:128, 64:128] = AT
    cs2_half = np.concatenate([C2T, S2T], axis=1)                 # [l, 128]
    cs2 = np.concatenate([cs2_half, cs2_half], axis=0)
    ICS = np.concatenate([C1T, -S1T], axis=0)
    return (khbd.astype(ml_dtypes.bfloat16),
            cs2.astype(ml_dtypes.bfloat16),
            np.ascontiguousarray(ICS.astype(ml_dtypes.bfloat16)))


_NC_CACHE = {}


def _build_nc(apply_gb):
    nc = bass.Bass(trn_type="TRN2")

    xa_d = nc.dram_tensor("xra", [128, 4096], BF16, kind="ExternalInput")
    xb_d = nc.dram_tensor("xrb", [128, 4096], BF16, kind="ExternalInput")
    kh_d = nc.dram_tensor("kh", [128, 128], BF16, kind="ExternalInput")
    cs_d = nc.dram_tensor("cs", [128, 128], BF16, kind="ExternalInput")
    ics_d = nc.dram_tensor("ics", [128, 64], BF16, kind="ExternalInput")
    w_d = nc.dram_tensor("weff", [128, 8192], BF16, kind="ExternalInput")
    pjt_d = nc.dram_tensor("pjt", [128, 512], BF16, kind="ExternalInput")
    gb_d = nc.dram_tensor("gb", [2, 256], F32, kind="ExternalInput")
    y_d = nc.dram_tensor("y", [4096, 256], BF16, kind="ExternalOutput")

    with TileContext(nc) as tc:
        with (
            tc.tile_pool(name="consts", bufs=1) as consts,
            tc.tile_pool(name="wf", bufs=1) as wf,
            tc.tile_pool(name="pAx", bufs=1) as pAx,
            tc.tile_pool(name="pBx", bufs=1) as pBx,
            tc.tile_pool(name="pAt", bufs=1) as pAt,
            tc.tile_pool(name="pBt", bufs=1) as pBt,
            tc.tile_pool(name="pT2", bufs=1) as pT2,
            tc.tile_pool(name="pZ", bufs=1) as pZ,
            tc.tile_pool(name="pU", bufs=1) as pU,
            tc.tile_pool(name="pUs", bufs=1) as pUs,
            tc.tile_pool(name="pY", bufs=1) as pY,
            tc.tile_pool(name="dramp", bufs=1, space="DRAM") as dramp,
            tc.tile_pool(name="ps", bufs=5, space="PSUM") as ps,
            tc.tile_pool(name="psy", bufs=3, space="PSUM") as psy,
            tc.tile_pool(name="small", bufs=8) as small,
        ):
            # ---- constants (gpsimd queue; x loads get sync/scalar) ----
            khbd = consts.tile([128, 128], BF16, tag="khbd")
            cs2 = consts.tile([128, 128], BF16, tag="cs2")
            ics = consts.tile([128, 64], BF16, tag="ics")
            pjt = consts.tile([128, 512], BF16, tag="pjt")
            nc.gpsimd.dma_start(out=khbd[:], in_=kh_d[:])
            nc.gpsimd.dma_start(out=cs2[:], in_=cs_d[:])
            nc.gpsimd.dma_start(out=ics[:], in_=ics_d[:])
            nc.gpsimd.dma_start(out=pjt[:], in_=pjt_d[:])
            eps = consts.tile([128, 1], F32, tag="eps")
            nc.vector.memset(eps[:], 1e-5)
            weff = wf.tile([128, 8192], BF16, tag="wf")
            nc.gpsimd.dma_start(out=weff[:], in_=w_d[:])
            if apply_gb:
                gt = consts.tile([128, 256], F32, tag="gt")
                bt = consts.tile([128, 256], F32, tag="bt")
                gb_ap = gb_d.ap()
                g_b = bass.AP(tensor=gb_ap.tensor, offset=0, ap=[[0, 128], [1, 256]])
                b_b = bass.AP(tensor=gb_ap.tensor, offset=256, ap=[[0, 128], [1, 256]])
                nc.gpsimd.dma_start(out=gt[:], in_=g_b)
                nc.gpsimd.dma_start(out=bt[:], in_=b_b)

            # ---- load x (w-split pipes: A = w 0:32, B = w 32:64) ----
            XA = pAx.tile([128, 4096], BF16, tag="pAx", name="XA")
            XB = pBx.tile([128, 4096], BF16, tag="pBx", name="XB")
            for hh in range(2):
                hs = slice(hh * 2048, (hh + 1) * 2048)
                nc.sync.dma_start(out=XA[:, hs], in_=xa_d[:, hs])
            for hh in range(2):
                hs = slice(hh * 2048, (hh + 1) * 2048)
                nc.scalar.dma_start(out=XB[:, hs], in_=xb_d[:, hs])

            # ---- S2: row DCT per pipe; T1[(wq,k),(w16,c256)] ----
            def s2(Xt, T1t):
                for j in range(8):
                    sl = slice(j * 512, (j + 1) * 512)
                    pt = ps.tile([128, 512], F32, tag="ps")
                    nc.tensor.matmul(pt[:], khbd[:], Xt[:, sl],
                                     start=True, stop=True)
                    eng = nc.vector.tensor_copy if j % 2 == 0 else nc.scalar.copy
                    eng(T1t[:, sl], pt[:])

            T1A = pAt.tile([128, 4096], BF16, tag="pAt", name="T1A")
            T1B = pBt.tile([128, 4096], BF16, tag="pBt", name="T1B")
            s2(XA, T1A)
            s2(XB, T1B)

            # ---- P1: k<->w pivot through DRAM, 512B runs ----
            # D1[w, (k64, c256)]; T2[(kg,w64), (k32,c256)]
            D1 = dramp.tile([64, 16384], BF16, tag="d1")
            D1v = D1[:].rearrange("w (k c) -> k w c", c=256)
            T2 = pT2.tile([128, 8192], BF16, tag="pT2", name="T2")
            for wq in range(2):
                nc.sync.dma_start(out=D1v[:, wq * 16:(wq + 1) * 16, :],
                                  in_=T1A[wq * 64:(wq + 1) * 64, :])
            for wq in range(2):
                nc.scalar.dma_start(out=D1v[:, 32 + wq * 16:32 + (wq + 1) * 16, :],
                                    in_=T1B[wq * 64:(wq + 1) * 64, :])
            for kg in range(2):
                ks = slice(kg * 8192, (kg + 1) * 8192)
                nc.sync.dma_start(out=T2[kg * 64:kg * 64 + 32, :], in_=D1[0:32, ks])
                nc.scalar.dma_start(out=T2[kg * 64 + 32:kg * 64 + 64, :],
                                    in_=D1[32:64, ks])

            # ---- S4: col DCT + W_eff; Z[(kg,l),(k32,c256)] ----
            Zp = pZ.tile([128, 8192], BF16, tag="pZ", name="Zp")
            for j in range(16):
                sl = slice(j * 512, (j + 1) * 512)
                pt = ps.tile([128, 512], F32, tag="ps")
                nc.tensor.matmul(pt[:], khbd[:], T2[:, sl], start=True, stop=True)
                nc.vector.tensor_mul(Zp[:, sl], pt[:], weff[:, sl])

            # ---- S5: inverse-l (cos & sin); U2s[(cs,n),(kg,k32,c256)] ----
            U2s = pU.tile([128, 16384], BF16, tag="pU", name="U2s")
            D2 = dramp.tile([128, 16384], BF16, tag="d2", name="D2")
            for kg in range(2):
                off = kg * 64
                for j in range(16):
                    sl = slice(j * 512, (j + 1) * 512)
                    pt = ps.tile([128, 512], F32, tag="ps")
                    nc.tensor.matmul(pt[:], cs2[off:off + 64, :],
                                     Zp[off:off + 64, sl], start=True, stop=True)
                    dsl = slice(kg * 8192 + j * 512, kg * 8192 + (j + 1) * 512)
                    eng = nc.vector.tensor_copy if j % 2 == 0 else nc.scalar.copy
                    eng(U2s[:, dsl], pt[:])

                # P2 stores for this kg as soon as its drains land
                for cs in range(2):
                    r0 = cs * 64 + kg * 32
                    dst = D2[r0:r0 + 32, :].rearrange("k (n c) -> n k c", c=256)
                    io = nc.sync if cs == 0 else nc.scalar
                    io.dma_start(out=dst,
                                 in_=U2s[cs * 64:(cs + 1) * 64,
                                         kg * 8192:(kg + 1) * 8192])

            # ---- tail: per n-quarter: load pivot, inverse-k, proj, LN ----
            Ustk = pUs.tile([128, 16384], BF16, tag="pUs", name="Ustk")
            X01 = [
                pAx.tile([128, 4096], BF16, tag="pAx", name="X01_0"),
                pBx.tile([128, 4096], BF16, tag="pBx", name="X01_1"),
            ]
            Yraw = pY.tile([128, 8192], BF16, tag="pY", name="Yraw")
            mvall = small.tile([128, 64], F32, tag="mvall")
            rstdall = small.tile([128, 32], F32, tag="rstdall")
            nmrall = small.tile([128, 32], F32, tag="nmrall")
            mvv = mvall[:].rearrange("p (t x) -> p t x", x=2)
            yv = y_d[:].rearrange("(t r) d -> r t d", r=128)

            for q in range(4):
                qs = slice(q * 4096, (q + 1) * 4096)
                io = nc.sync if q % 2 == 0 else nc.scalar
                io.dma_start(out=Ustk[:, qs], in_=D2[:, qs])
                for chalf in range(2):
                    for g in (2 * q, 2 * q + 1):
                        pt = ps.tile([128, 512], F32, tag="ps")
                        for nn in range(8):
                            n0 = 8 * g + nn
                            cofs = n0 * 256 + chalf * 128
                            nc.tensor.matmul(
                                pt[:, nn * 64:(nn + 1) * 64],
                                Ustk[:, cofs:cofs + 128],
                                ics[:], start=True, stop=True,
                            )
                        eng = nc.vector.tensor_copy if g % 2 == 0 else nc.scalar.copy
                        eng(X01[chalf][:, g * 512:(g + 1) * 512], pt[:])
                for t2 in range(8 * q, 8 * q + 8):
                    pty = psy.tile([128, 256], F32, tag="psy")
                    nc.tensor.matmul(pty[:], X01[0][:, t2 * 128:(t2 + 1) * 128],
                                     pjt[:, 0:256], start=True, stop=False)
                    nc.tensor.matmul(pty[:], X01[1][:, t2 * 128:(t2 + 1) * 128],
                                     pjt[:, 256:512], start=False, stop=True)
                    stats = small.tile([128, 6], F32, tag="stats")
                    nc.vector.bn_stats(out=stats[:], in_=pty[:])
                    nc.vector.bn_aggr(out=mvall[:, t2 * 2:(t2 + 1) * 2], in_=stats[:])
                    eng = nc.scalar.copy if t2 % 2 == 0 else nc.vector.tensor_copy
                    eng(Yraw[:, t2 * 256:(t2 + 1) * 256], pty[:])
                gs = slice(q * 8, q * 8 + 8)
                # std = sqrt(var + eps); rstd = 1/std; nmr = -mu*rstd
                nc.scalar.activation(out=rstdall[:, gs],
                                     in_=mvv[:, gs, 1], func=ACTF.Sqrt,
                                     bias=eps[:], scale=1.0)
                nc.vector.reciprocal(rstdall[:, gs], rstdall[:, gs])
                nc.vector.tensor_tensor(out=nmrall[:, gs], in0=mvv[:, gs, 0],
                                        in1=rstdall[:, gs], op=ALU.mult)
                nc.vector.tensor_scalar_mul(nmrall[:, gs], nmrall[:, gs], -1.0)
                for t3 in range(q * 8, q * 8 + 8):
                    ysl = slice(t3 * 256, (t3 + 1) * 256)
                    if t3 % 2 == 0:
                        nc.gpsimd.tensor_scalar(
                            out=Yraw[:, ysl], in0=Yraw[:, ysl],
                            scalar1=rstdall[:, t3:t3 + 1],
                            scalar2=nmrall[:, t3:t3 + 1],
                            op0=ALU.mult, op1=ALU.add,
                        )
                    else:
                        nc.scalar.activation(
                            out=Yraw[:, ysl], in_=Yraw[:, ysl],
                            func=ACTF.Identity,
                            bias=nmrall[:, t3:t3 + 1],
                            scale=rstdall[:, t3:t3 + 1],
                        )
                    if apply_gb:
                        nc.vector.tensor_mul(Yraw[:, ysl], Yraw[:, ysl], gt[:])
                        nc.gpsimd.tensor_add(Yraw[:, ysl], Yraw[:, ysl], bt[:])
                nc.sync.dma_start(out=yv[:, q * 8:(q + 1) * 8, :],
                                  in_=Yraw[:, q * 2048:(q + 1) * 2048])

    _split_multi_waits(nc)
    return nc


def _get_nc(apply_gb):
    key = bool(apply_gb)
    if key not in _NC_CACHE:
        _NC_CACHE[key] = _build_nc(key)
    return _NC_CACHE[key]


def _make_inputs(x, W_low, W_mid, W_high, proj_w, ln_g, ln_b):
    khbd, cs2, ICS = _host_matrices()

    W_eff = W_high[0].copy()
    W_eff[:32, :32] += W_mid[0]
    W_eff[:16, :16] += W_low[0]
    # weff[(kg,l), (k32,c256)]
    weff = np.ascontiguousarray(
        W_eff.reshape(2, 32, 64, 256).transpose(0, 2, 1, 3)
        .reshape(128, 8192).astype(ml_dtypes.bfloat16)
    )

    pjt = np.zeros((128, 512), ml_dtypes.bfloat16)
    pjt[:, :256] = proj_w.T[:128]
    pjt[:, 256:] = proj_w.T[128:]

    gb = np.stack([ln_g, ln_b]).astype(np.float32)
    consts = {"kh": khbd, "cs": cs2, "ics": ICS,
              "weff": weff, "pjt": pjt, "gb": gb}

    in_maps = []
    for b in range(B):
        m = dict(consts)
        x2d = x[b].reshape(64, 64, 256)
        for P, name in ((0, "xra"), (1, "xrb")):
            xp = x2d[:, 32 * P:32 * (P + 1), :]                   # [h, w32, c]
            m[name] = np.ascontiguousarray(
                xp.reshape(64, 2, 16, 256).transpose(1, 0, 2, 3)
                .reshape(128, 4096).astype(ml_dtypes.bfloat16)
            )
        in_maps.append(m)
    return in_maps


def kernel(x, W_low, W_mid, W_high, proj_w, ln_g, ln_b):
    x = np.ascontiguousarray(np.asarray(x, dtype=np.float32))
    W_low = np.asarray(W_low, dtype=np.float32)
    W_mid = np.asarray(W_mid, dtype=np.float32)
    W_high = np.asarray(W_high, dtype=np.float32)
    proj_w = np.asarray(proj_w, dtype=np.float32)
    ln_g = np.asarray(ln_g, dtype=np.float32)
    ln_b = np.asarray(ln_b, dtype=np.float32)

    apply_gb = not (np.all(ln_g == 1.0) and np.all(ln_b == 0.0))
    in_maps = _make_inputs(x, W_low, W_mid, W_high, proj_w, ln_g, ln_b)
    nc = _get_nc(apply_gb)
    res = run_bass_kernel_spmd(nc, in_maps, core_ids=list(range(B)))

    out = np.empty((B, N, C), np.float32)
    for b in range(B):
        yc = np.asarray(res.results[b]["y"]).astype(np.float32)
        out[b] = yc.reshape(64, 64, 256).transpose(1, 0, 2).reshape(4096, 256)
    return out
